# revision 2
# baseline (speedup 1.0000x reference)
"""GCN layer on 8 trn2 NeuronCores (Bass/Tile).

Reference computation:
    h = segment_sum(feature[src], dst, 100000) @ W.T + b

Strategy (1D dst partition, per the standard SpMM graph partitioning):
  - Partition dst nodes across 8 cores (12500 each); each core owns the
    edges whose dst lands in its range.
  - On the host: bucket each core's edges by 128-node dst window, pad each
    window's edge list to a multiple of 128, and lay out per-edge src
    indices / window-local dst as [128, T] arrays (tile t = column t).
  - On device, per 128-edge tile:
      gather M[e,d] = feature[src[e]]          (SWDGE indirect DMA)
      S_T[e,n]     = (dst_loc[e] == n)         (DVE is_equal vs iota)
      HT[d,n]     += M.T @ S_T                 (TensorE, PSUM-accumulated
                                                across the window's tiles)
  - Per window: OUT[n,o] = HT.T @ W.T + b      (TensorE + DVE bias add)
  - The gather is descriptor-rate-bound on SWDGE (~11 ns/row); everything
    else overlaps underneath it.

Self-contained: hardcodes shapes from the problem spec.
"""

import sys
import types

import numpy as np

N_NODES = 100000
N_EDGES = 1600000
D = 128
CORES = 8
NPC = N_NODES // CORES        # 12500 nodes per core
WIN = 128                     # dst nodes per window
NWIN = (NPC + WIN - 1) // WIN  # 98 windows per core
OUT_ROWS = NWIN * WIN         # 12544 (last 44 rows trimmed on host)
K_ST = 8                      # edge tiles per S_T build group
PAD_DLOC = 200.0              # window-local dst for padding edges (matches no iota)


def _install_axon_profile_hook():
    """bass_utils' trace path imports antenv.axon_hooks, which this image
    lacks; recreate it around trn_boot's ctypes NTFF driver."""
    if "antenv.axon_hooks" in sys.modules:
        return
    try:
        import antenv
    except ImportError:
        return
    hooks = types.ModuleType("antenv.axon_hooks")
    holder = {}
    hooks.set_axon_ntff_profile_hook = lambda h: holder.__setitem__("h", h)
    hooks.get_axon_ntff_profile_hook = lambda: holder.get("h")
    sys.modules["antenv.axon_hooks"] = hooks
    antenv.axon_hooks = hooks
    try:
        from trn_agent_boot.trn_boot import _ntff_profile_via_ctypes

        hooks.set_axon_ntff_profile_hook(
            _ntff_profile_via_ctypes("/opt/axon/libaxon_pjrt.so")
        )
    except Exception:
        pass


def _patch_tile_drain():
    """This walrus build rejects >1 sync-wait per instruction. Split every
    multi-wait (including the kernel-tail drain's) onto single-wait nops."""
    import concourse.mybir as mybir
    import concourse.tile as tile
    from concourse.vector_clock import ScopedClock

    if getattr(tile.TileContext, "_mw_split_patched", False):
        return

    def _split_multiwaits(nc):
        cur = nc.cur_bb.bb
        for bb in nc.m.functions[0].blocks:
            insts = list(bb.instructions)
            if not any(
                inst.sync_info is not None and len(inst.sync_info.on_wait) > 1
                for inst in insts
            ):
                continue
            rebuilt = []
            for inst in insts:
                si = inst.sync_info
                if si is not None and len(si.on_wait) > 1:
                    waits = list(si.on_wait)
                    si.on_wait = [waits[-1]]
                    for wt in waits[:-1]:
                        nop = nc.engines[inst.engine].nop(
                            nofuse=True, hint="mw_split"
                        )
                        cur.instructions.pop()  # un-append; place manually
                        nop.ins.sync_info = mybir.SyncInfo(
                            on_wait=[wt], on_update=[]
                        )
                        rebuilt.append(nop.ins)
                rebuilt.append(inst)
            bb.instructions[:] = rebuilt

    def patched(self, tick_clock, wait_clock):
        drain_inst = self.nc.sync.drain()
        wait_clock.add_sem_waits(
            drain_inst.ins, ScopedClock({None: tick_clock.global_clock})
        )
        _split_multiwaits(self.nc)
        self.nc.all_engine_barrier()
        popped = self.nc._tile_sem_poison_stack.pop()
        assert popped is self._sem_poison
        self.nc.clear_and_free_semaphores(list(self.sems.allocated().values()))
        self.nc.all_engine_barrier()

    tile.TileContext._drain_and_barrier = patched
    tile.TileContext._mw_split_patched = True


def _prep_edges(src, dst):
    """Bucket edges by (core, window); pad each window to 128-multiples with
    a tile count shared across cores (SPMD needs one program).

    Returns (esrc [8,128,T] int32, edloc [8,128,T] f32, tiles_per_w [NWIN])."""
    src = np.asarray(src).astype(np.int64, copy=False)
    dst = np.asarray(dst).astype(np.int64, copy=False)
    core = dst // NPC
    nloc = dst - core * NPC
    w = nloc >> 7
    dloc = (nloc & 127).astype(np.float32)
    cw = core * NWIN + w

    counts = np.bincount(cw, minlength=CORES * NWIN).reshape(CORES, NWIN)
    tiles_per_w = np.maximum(1, -(-counts // WIN)).max(axis=0)  # ceil, >=1
    toff = np.zeros(NWIN + 1, np.int64)
    np.cumsum(tiles_per_w, out=toff[1:])
    T = int(toff[-1])

    esrc = np.zeros((CORES, WIN, T), np.int32)
    edloc = np.full((CORES, WIN, T), PAD_DLOC, np.float32)

    order = np.argsort(cw, kind="stable")
    sorted_cw = cw[order]
    grp_start = np.searchsorted(sorted_cw, np.arange(CORES * NWIN))
    ranks = np.arange(len(order)) - grp_start[sorted_cw]
    c_s = core[order]
    w_s = w[order]
    t_global = toff[w_s] + (ranks >> 7)
    p = ranks & 127
    esrc[c_s, p, t_global] = src[order].astype(np.int32)
    edloc[c_s, p, t_global] = dloc[order]
    return esrc, edloc, tiles_per_w.astype(np.int64), toff


def _build_program(tiles_per_w, toff, T):
    import concourse.bass as bass
    import concourse.mybir as mybir
    import concourse.tile as tile

    f32 = mybir.dt.float32
    i32 = mybir.dt.int32

    nc = bass.Bass()
    feat = nc.declare_dram_parameter("feat", [N_NODES, D], f32, isOutput=False)
    esrc = nc.declare_dram_parameter("esrc", [WIN, T], i32, isOutput=False)
    edloc = nc.declare_dram_parameter("edloc", [WIN, T], f32, isOutput=False)
    iota = nc.declare_dram_parameter("iota", [WIN, K_ST * WIN], f32, isOutput=False)
    wt = nc.declare_dram_parameter("wt", [D, D], f32, isOutput=False)
    biasf = nc.declare_dram_parameter("biasf", [WIN, D], f32, isOutput=False)
    outp = nc.declare_dram_parameter("outp", [OUT_ROWS, D], f32, isOutput=True)

    with tile.TileContext(nc) as tc:
        with (
            tc.tile_pool(name="const", bufs=1) as cpool,
            tc.tile_pool(name="gat", bufs=24) as gpool,
            tc.tile_pool(name="st", bufs=4) as stpool,
            tc.tile_pool(name="htps", bufs=2, space="PSUM") as htps,
            tc.tile_pool(name="outps", bufs=2, space="PSUM") as outps,
            tc.tile_pool(name="htsb", bufs=3) as htsb,
            tc.tile_pool(name="osb", bufs=3) as osb,
        ):
            esrc_t = cpool.tile([WIN, T], i32)
            edloc_t = cpool.tile([WIN, T], f32)
            iota_t = cpool.tile([WIN, K_ST * WIN], f32)
            wt_t = cpool.tile([D, D], f32)
            bias_t = cpool.tile([WIN, D], f32)
            nc.sync.dma_start(out=esrc_t[:], in_=esrc[:])
            nc.sync.dma_start(out=edloc_t[:], in_=edloc[:])
            nc.sync.dma_start(out=iota_t[:], in_=iota[:])
            nc.sync.dma_start(out=wt_t[:], in_=wt[:])
            nc.sync.dma_start(out=bias_t[:], in_=biasf[:])

            st_tiles = {}

            def st_for(t):
                grp = t // K_ST
                if grp not in st_tiles:
                    k = min(K_ST, T - grp * K_ST)
                    st = stpool.tile([WIN, K_ST * WIN], f32)
                    nc.vector.tensor_tensor(
                        out=st[:, : k * WIN].rearrange("p (k n) -> p k n", k=k),
                        in0=edloc_t[:, grp * K_ST : grp * K_ST + k].to_broadcast(
                            [WIN, k, WIN]
                        ),
                        in1=iota_t[:, : k * WIN].rearrange("p (k n) -> p k n", k=k),
                        op=mybir.AluOpType.is_equal,
                    )
                    st_tiles[grp] = st
                return st_tiles[grp], t - grp * K_ST

            for w in range(NWIN):
                m = int(tiles_per_w[w])
                ht = htps.tile([D, WIN], f32, space="PSUM")
                for j in range(m):
                    t = int(toff[w]) + j
                    g = gpool.tile([WIN, D], f32)
                    nc.gpsimd.indirect_dma_start(
                        out=g[:],
                        out_offset=None,
                        in_=feat[:],
                        in_offset=bass.IndirectOffsetOnAxis(
                            ap=esrc_t[:, t : t + 1], axis=0
                        ),
                    )
                    st, col = st_for(t)
                    nc.tensor.matmul(
                        out=ht[:],
                        lhsT=g[:],
                        rhs=st[:, col * WIN : (col + 1) * WIN],
                        start=(j == 0),
                        stop=(j == m - 1),
                    )
                hts = htsb.tile([D, WIN], f32)
                nc.vector.tensor_copy(out=hts[:], in_=ht[:])
                ops = outps.tile([WIN, D], f32, space="PSUM")
                nc.tensor.matmul(out=ops[:], lhsT=hts[:], rhs=wt_t[:], start=True, stop=True)
                ot = osb.tile([WIN, D], f32)
                nc.vector.tensor_tensor(
                    out=ot[:], in0=ops[:], in1=bias_t[:], op=mybir.AluOpType.add
                )
                nc.sync.dma_start(out=outp[w * WIN : (w + 1) * WIN, :], in_=ot[:])
    return nc


def kernel(feature, src, dst, W, b):
    _install_axon_profile_hook()
    _patch_tile_drain()
    from concourse.bass_utils import run_bass_kernel_spmd

    feature = np.ascontiguousarray(np.asarray(feature, dtype=np.float32))
    W = np.asarray(W, dtype=np.float32)
    b = np.asarray(b, dtype=np.float32)

    esrc, edloc, tiles_per_w, toff = _prep_edges(src, dst)
    T = int(toff[-1])

    nc = _build_program(tiles_per_w, toff, T)

    iota = np.tile(np.arange(WIN, dtype=np.float32), (WIN, K_ST))
    wt = np.ascontiguousarray(W.T)
    biasf = np.tile(b[None, :], (WIN, 1)).astype(np.float32)

    in_maps = [
        {
            "feat": feature,
            "esrc": np.ascontiguousarray(esrc[c]),
            "edloc": np.ascontiguousarray(edloc[c]),
            "iota": iota,
            "wt": wt,
            "biasf": biasf,
        }
        for c in range(CORES)
    ]
    res = run_bass_kernel_spmd(nc, in_maps, list(range(CORES)))
    out = np.concatenate(
        [res.results[c]["outp"][:NPC] for c in range(CORES)], axis=0
    )
    return out


# revision 5
# speedup vs baseline: 1.0612x; 1.0612x over previous
"""GCN layer on 8 trn2 NeuronCores (Bass/Tile).

Reference computation:
    h = segment_sum(feature[src], dst, 100000) @ W.T + b

Strategy (1D dst partition, per the standard SpMM graph partitioning):
  - Partition dst nodes across 8 cores (12500 each); each core owns the
    edges whose dst lands in its range.
  - On the host: bucket each core's edges by 128-node dst window, pad each
    window's edge list to a multiple of 128, and lay out per-edge src
    indices / window-local dst as [128, T] arrays (tile t = column t).
  - On device, per 128-edge tile:
      gather M[e,d] = feature[src[e]]          (SWDGE indirect DMA)
      S_T[e,n]     = (dst_loc[e] == n)         (DVE is_equal vs iota)
      HT[d,n]     += M.T @ S_T                 (TensorE, PSUM-accumulated
                                                across the window's tiles)
  - Per window: OUT[n,o] = HT.T @ W.T + b      (TensorE + DVE bias add)
  - The gather is descriptor-rate-bound on SWDGE (~11 ns/row); everything
    else overlaps underneath it.

Self-contained: hardcodes shapes from the problem spec.
"""

import sys
import types

import numpy as np

N_NODES = 100000
N_EDGES = 1600000
D = 128
CORES = 8
NPC = N_NODES // CORES        # 12500 nodes per core
WIN = 128                     # dst nodes per window
NWIN = (NPC + WIN - 1) // WIN  # 98 windows per core
OUT_ROWS = NWIN * WIN         # 12544 (last 44 rows trimmed on host)
K_ST = 8                      # edge tiles per S_T build group
PAD_DLOC = 200.0              # window-local dst for padding edges (matches no iota)


def _install_axon_profile_hook():
    """bass_utils' trace path imports antenv.axon_hooks, which this image
    lacks; recreate it around trn_boot's ctypes NTFF driver."""
    if "antenv.axon_hooks" in sys.modules:
        return
    try:
        import antenv
    except ImportError:
        return
    hooks = types.ModuleType("antenv.axon_hooks")
    holder = {}
    hooks.set_axon_ntff_profile_hook = lambda h: holder.__setitem__("h", h)
    hooks.get_axon_ntff_profile_hook = lambda: holder.get("h")
    sys.modules["antenv.axon_hooks"] = hooks
    antenv.axon_hooks = hooks
    try:
        from trn_agent_boot.trn_boot import _ntff_profile_via_ctypes

        hooks.set_axon_ntff_profile_hook(
            _ntff_profile_via_ctypes("/opt/axon/libaxon_pjrt.so")
        )
    except Exception:
        pass


def _patch_tile_drain():
    """This walrus build rejects >1 sync-wait per instruction. Split every
    multi-wait (including the kernel-tail drain's) onto single-wait nops."""
    import concourse.mybir as mybir
    import concourse.tile as tile
    from concourse.vector_clock import ScopedClock

    if getattr(tile.TileContext, "_mw_split_patched", False):
        return

    def _split_multiwaits(nc):
        cur = nc.cur_bb.bb
        for bb in nc.m.functions[0].blocks:
            insts = list(bb.instructions)
            if not any(
                inst.sync_info is not None and len(inst.sync_info.on_wait) > 1
                for inst in insts
            ):
                continue
            rebuilt = []
            for inst in insts:
                si = inst.sync_info
                if si is not None and len(si.on_wait) > 1:
                    waits = list(si.on_wait)
                    si.on_wait = [waits[-1]]
                    for wt in waits[:-1]:
                        nop = nc.engines[inst.engine].nop(
                            nofuse=True, hint="mw_split"
                        )
                        cur.instructions.pop()  # un-append; place manually
                        nop.ins.sync_info = mybir.SyncInfo(
                            on_wait=[wt], on_update=[]
                        )
                        rebuilt.append(nop.ins)
                rebuilt.append(inst)
            bb.instructions[:] = rebuilt

    def patched(self, tick_clock, wait_clock):
        drain_inst = self.nc.sync.drain()
        wait_clock.add_sem_waits(
            drain_inst.ins, ScopedClock({None: tick_clock.global_clock})
        )
        _split_multiwaits(self.nc)
        self.nc.all_engine_barrier()
        popped = self.nc._tile_sem_poison_stack.pop()
        assert popped is self._sem_poison
        self.nc.clear_and_free_semaphores(list(self.sems.allocated().values()))
        self.nc.all_engine_barrier()

    tile.TileContext._drain_and_barrier = patched
    tile.TileContext._mw_split_patched = True


def _assign_nodes(dst):
    """Degree-balance dst nodes across the 8*98 (core,window) bins so every
    bin's edge count stays ~= E/bins (minimizes the shared SPMD tile count).
    Greedy: deal degree-sorted nodes one per bin per round, least-loaded
    bins first. Each bin receives <=1 node per round => <=128 nodes total.

    Returns (node_bin [N], node_slot [N]) with slot in [0,128)."""
    nbins = CORES * NWIN
    deg = np.bincount(dst, minlength=N_NODES)
    order = np.argsort(-deg, kind="stable")
    loads = np.zeros(nbins, np.int64)
    node_bin = np.empty(N_NODES, np.int32)
    node_slot = np.empty(N_NODES, np.int32)
    for r in range(-(-N_NODES // nbins)):
        chunk = order[r * nbins : (r + 1) * nbins]
        bins = np.argsort(loads, kind="stable")[: len(chunk)]
        node_bin[chunk] = bins
        node_slot[chunk] = r
        loads[bins] += deg[chunk]
    return node_bin, node_slot


def _prep_edges(src, dst):
    """Assign dst nodes to balanced (core,window,slot) positions, bucket
    edges by (core, window), pad each window to 128-multiples with a tile
    count shared across cores (SPMD needs one program).

    Returns (esrc [8,128,T] i32, edloc [8,128,T] f32, tiles_per_w, toff,
    perm) where perm maps original node id -> row of the concatenated
    device output."""
    src = np.asarray(src).astype(np.int64, copy=False)
    dst = np.asarray(dst).astype(np.int64, copy=False)

    node_bin, node_slot = _assign_nodes(dst)
    perm = node_bin.astype(np.int64) * WIN + node_slot  # row in [8*12544]

    ebin = node_bin[dst]                   # (core*NWIN + window) per edge
    dloc = node_slot[dst].astype(np.float32)
    core = ebin // NWIN
    w = ebin % NWIN

    counts = np.bincount(ebin, minlength=CORES * NWIN).reshape(CORES, NWIN)
    tiles_per_w = np.maximum(1, -(-counts // WIN)).max(axis=0)  # ceil, >=1
    toff = np.zeros(NWIN + 1, np.int64)
    np.cumsum(tiles_per_w, out=toff[1:])
    T = int(toff[-1])

    esrc = np.zeros((CORES, WIN, T), np.int32)
    edloc = np.full((CORES, WIN, T), PAD_DLOC, np.float32)

    order = np.argsort(ebin, kind="stable")
    sorted_cw = ebin[order]
    grp_start = np.searchsorted(sorted_cw, np.arange(CORES * NWIN))
    ranks = np.arange(len(order)) - grp_start[sorted_cw]
    c_s = core[order]
    w_s = w[order]
    t_global = toff[w_s] + (ranks >> 7)
    p = ranks & 127
    esrc[c_s, p, t_global] = src[order].astype(np.int32)
    edloc[c_s, p, t_global] = dloc[order]
    return esrc, edloc, tiles_per_w.astype(np.int64), toff, perm


def _build_program(tiles_per_w, toff, T):
    import concourse.bass as bass
    import concourse.mybir as mybir
    import concourse.tile as tile

    f32 = mybir.dt.float32
    i32 = mybir.dt.int32

    nc = bass.Bass()
    feat = nc.declare_dram_parameter("feat", [N_NODES, D], f32, isOutput=False)
    esrc = nc.declare_dram_parameter("esrc", [WIN, T], i32, isOutput=False)
    edloc = nc.declare_dram_parameter("edloc", [WIN, T], f32, isOutput=False)
    iota = nc.declare_dram_parameter("iota", [WIN, K_ST * WIN], f32, isOutput=False)
    wt = nc.declare_dram_parameter("wt", [D, D], f32, isOutput=False)
    biasf = nc.declare_dram_parameter("biasf", [WIN, D], f32, isOutput=False)
    outp = nc.declare_dram_parameter("outp", [OUT_ROWS, D], f32, isOutput=True)

    with tile.TileContext(nc) as tc:
        with (
            tc.tile_pool(name="const", bufs=1) as cpool,
            tc.tile_pool(name="gat", bufs=24) as gpool,
            tc.tile_pool(name="st", bufs=4) as stpool,
            tc.tile_pool(name="htps", bufs=2, space="PSUM") as htps,
            tc.tile_pool(name="outps", bufs=2, space="PSUM") as outps,
            tc.tile_pool(name="htsb", bufs=3) as htsb,
            tc.tile_pool(name="osb", bufs=3) as osb,
        ):
            esrc_t = cpool.tile([WIN, T], i32)
            edloc_t = cpool.tile([WIN, T], f32)
            iota_t = cpool.tile([WIN, K_ST * WIN], f32)
            wt_t = cpool.tile([D, D], f32)
            bias_t = cpool.tile([WIN, D], f32)
            nc.sync.dma_start(out=esrc_t[:], in_=esrc[:])
            nc.sync.dma_start(out=edloc_t[:], in_=edloc[:])
            nc.sync.dma_start(out=iota_t[:], in_=iota[:])
            nc.sync.dma_start(out=wt_t[:], in_=wt[:])
            nc.sync.dma_start(out=bias_t[:], in_=biasf[:])

            st_tiles = {}

            def st_for(t):
                grp = t // K_ST
                if grp not in st_tiles:
                    k = min(K_ST, T - grp * K_ST)
                    st = stpool.tile([WIN, K_ST * WIN], f32)
                    nc.vector.tensor_tensor(
                        out=st[:, : k * WIN].rearrange("p (k n) -> p k n", k=k),
                        in0=edloc_t[:, grp * K_ST : grp * K_ST + k].to_broadcast(
                            [WIN, k, WIN]
                        ),
                        in1=iota_t[:, : k * WIN].rearrange("p (k n) -> p k n", k=k),
                        op=mybir.AluOpType.is_equal,
                    )
                    st_tiles[grp] = st
                return st_tiles[grp], t - grp * K_ST

            for w in range(NWIN):
                m = int(tiles_per_w[w])
                ht = htps.tile([D, WIN], f32, space="PSUM")
                for j in range(m):
                    t = int(toff[w]) + j
                    g = gpool.tile([WIN, D], f32)
                    nc.gpsimd.indirect_dma_start(
                        out=g[:],
                        out_offset=None,
                        in_=feat[:],
                        in_offset=bass.IndirectOffsetOnAxis(
                            ap=esrc_t[:, t : t + 1], axis=0
                        ),
                    )
                    st, col = st_for(t)
                    nc.tensor.matmul(
                        out=ht[:],
                        lhsT=g[:],
                        rhs=st[:, col * WIN : (col + 1) * WIN],
                        start=(j == 0),
                        stop=(j == m - 1),
                    )
                hts = htsb.tile([D, WIN], f32)
                nc.vector.tensor_copy(out=hts[:], in_=ht[:])
                ops = outps.tile([WIN, D], f32, space="PSUM")
                nc.tensor.matmul(out=ops[:], lhsT=hts[:], rhs=wt_t[:], start=True, stop=True)
                ot = osb.tile([WIN, D], f32)
                nc.vector.tensor_tensor(
                    out=ot[:], in0=ops[:], in1=bias_t[:], op=mybir.AluOpType.add
                )
                nc.sync.dma_start(out=outp[w * WIN : (w + 1) * WIN, :], in_=ot[:])
    return nc


def kernel(feature, src, dst, W, b):
    _install_axon_profile_hook()
    _patch_tile_drain()
    from concourse.bass_utils import run_bass_kernel_spmd

    feature = np.ascontiguousarray(np.asarray(feature, dtype=np.float32))
    W = np.asarray(W, dtype=np.float32)
    b = np.asarray(b, dtype=np.float32)

    esrc, edloc, tiles_per_w, toff, perm = _prep_edges(src, dst)
    T = int(toff[-1])

    nc = _build_program(tiles_per_w, toff, T)

    iota = np.tile(np.arange(WIN, dtype=np.float32), (WIN, K_ST))
    wt = np.ascontiguousarray(W.T)
    biasf = np.tile(b[None, :], (WIN, 1)).astype(np.float32)

    in_maps = [
        {
            "feat": feature,
            "esrc": np.ascontiguousarray(esrc[c]),
            "edloc": np.ascontiguousarray(edloc[c]),
            "iota": iota,
            "wt": wt,
            "biasf": biasf,
        }
        for c in range(CORES)
    ]
    res = run_bass_kernel_spmd(nc, in_maps, list(range(CORES)))
    allrows = np.concatenate(
        [res.results[c]["outp"] for c in range(CORES)], axis=0
    )
    return np.ascontiguousarray(allrows[perm])
